# revision 20
# baseline (speedup 1.0000x reference)
"""Trainium2 Bass kernel for nn_Attention_33036888441230.

Cross-attention transformer block, B=8 batch sharded 1-per-core across 8
NeuronCores (pure data parallel, no collectives).

reference math (per batch):
  xn = LN(x,g1,b1); yn = LN(y,g2,b2)
  q = (xn@wq+bq).view(N,H,64); k = (yn@wk+bk).view(M,H,64)
  v = (yn@wv+bv).view(M,H,E)
  a = softmax(q.k^T/8, axis=m)
  dx = einsum('hnm,mhe->ne', a, v)       # heads summed
  h = LN3(xn + dx); out1 = h + relu(h@w_in+b_in)@w_out + b_out
  returns (out1, yn)

Implementation notes:
  - the four big matmul groups (q/k projections, v projection, attention.V
    accumulate) run in fp8-e4m3 with DoubleRow perf mode: K=256 per
    instruction at 2 weight-rows/PE-cell, 2-4x bf16 throughput. Weights are
    pre-scaled by a power of two on host into fp8's sweet range; the descale
    rides for free on the PSUM-evacuation instruction (ACT scale / DVE
    tensor_scalar imm).
  - score matmuls (K=64 per head) and the MLP stay bf16; layer-norm /
    softmax-normalization / residual math in fp32.
  - softmax has no max-subtraction (scores provably tiny); row-sums come for
    free from a ones-column appended to the attention.V matmul; 1/rowsum is
    fused into the per-head accumulate (scalar_tensor_tensor).
  - softmax-of-rows sums to 1 => v-bias contributes sum_h bv_h, folded on host.
  - LN gains/biases are folded into adjacent weights on host; the residual /
    output copies get an on-device affine only when non-identity.
  - PSUM evacuations are split between ACT and DVE (Pool cannot read PSUM)
    to balance the two element-wise engines.
  - heads processed in pairs: even head in PE rows 0:64, odd head in rows
    64:128, score matmuls interleaved so both run concurrently in the array.
"""

import sys

if "/opt/trn_rl_repo" not in sys.path:
    sys.path.insert(0, "/opt/trn_rl_repo")

from contextlib import ExitStack

import numpy as np

import concourse.bass as bass
import concourse.mybir as mybir
import concourse.tile as tile
from concourse import bacc
from concourse.masks import make_identity

F32 = mybir.dt.float32
BF16 = mybir.dt.bfloat16
FP8 = mybir.dt.float8e4
OP = mybir.AluOpType
AF = mybir.ActivationFunctionType
DR = mybir.MatmulPerfMode.DoubleRow

P = 128
HD = 64
EPS = 1e-5
N_CORES = 8


def _chunks(n, c=512):
    return [(i, min(i + c, n)) for i in range(0, n, c)]


def build(T, E, H, aff=(False, False, False), s_q=4096.0, s_k=2048.0,
          s_v=2048.0, s_i=2048.0, s_o=2048.0):
    """Build the per-core Bass graph. T tokens, E embed, H heads (HD=64)."""
    TT, ET = T // P, E // P
    KP = ET // 2  # fp8 DoubleRow K-pair count for E-contractions
    TP = TT // 2  # fp8 DoubleRow K-pair count for T-contractions
    assert H * HD == E
    nc = bacc.Bacc("TRN2", num_devices=N_CORES)

    x_d = nc.dram_tensor("x", [T, E], F32, kind="ExternalInput").ap()
    y_d = nc.dram_tensor("y", [T, E], F32, kind="ExternalInput").ap()
    wq_d = nc.dram_tensor("wq", [E, E], FP8, kind="ExternalInput").ap()
    wk_d = nc.dram_tensor("wk", [E, E], FP8, kind="ExternalInput").ap()
    wv_d = nc.dram_tensor("wv", [H, E, E], FP8, kind="ExternalInput").ap()
    wi_d = nc.dram_tensor("w_in", [E, E], FP8, kind="ExternalInput").ap()
    wo_d = nc.dram_tensor("w_out", [E, E], FP8, kind="ExternalInput").ap()
    bq_d = nc.dram_tensor("bq", [E], F32, kind="ExternalInput").ap()
    bk_d = nc.dram_tensor("bk", [E], F32, kind="ExternalInput").ap()
    bvs_d = nc.dram_tensor("bvs", [E], BF16, kind="ExternalInput").ap()
    bi_d = nc.dram_tensor("b_in", [E], F32, kind="ExternalInput").ap()
    bo_d = nc.dram_tensor("b_out", [E], BF16, kind="ExternalInput").ap()
    aff_d = {}
    for i, need in enumerate(aff):
        if need:
            aff_d[i] = (
                nc.dram_tensor(f"affg{i}", [E], BF16, kind="ExternalInput").ap(),
                nc.dram_tensor(f"affb{i}", [E], BF16, kind="ExternalInput").ap(),
            )
    o1_d = nc.dram_tensor("o1", [T, E], F32, kind="ExternalOutput").ap()
    oyn_d = nc.dram_tensor("oyn", [T, E], F32, kind="ExternalOutput").ap()

    with tile.TileContext(nc) as tc, ExitStack() as ctx:
        persist = ctx.enter_context(tc.tile_pool(name="persist", bufs=1))
        ps_big = ctx.enter_context(tc.tile_pool(name="psb", bufs=2, space="PSUM"))
        stp = ctx.enter_context(tc.tile_pool(name="stats", bufs=6))

        xn = persist.tile([P, TT, E], F32, tag="xn")
        acc = persist.tile([P, TT, E], F32, tag="acc")
        ynT = persist.tile([P, ET, T], FP8, tag="ynT")
        BV = persist.tile([P, E], F32, tag="BV")
        BO = persist.tile([P, E], F32, tag="BO")
        ident = persist.tile([P, P], F32, tag="ident")
        ones_r = persist.tile([1, P], BF16, tag="ones")
        bq_sb = persist.tile([P, ET], F32, tag="bq")
        bk_sb = persist.tile([P, ET], F32, tag="bk")
        bi_sb = persist.tile([P, ET], F32, tag="bi")
        bo_row = persist.tile([1, E], BF16, tag="bo")
        bvs_row = persist.tile([1, E], BF16, tag="bvs")
        eps_col = persist.tile([P, 1], F32, tag="eps")

        make_identity(nc, ident[:])
        nc.vector.memset(ones_r[:], 1.0)
        nc.vector.memset(eps_col[:], EPS)
        nc.sync.dma_start(bq_sb[:], bq_d.rearrange("(a p) -> p a", p=P))
        nc.sync.dma_start(bk_sb[:], bk_d.rearrange("(a p) -> p a", p=P))
        nc.sync.dma_start(bi_sb[:], bi_d.rearrange("(a p) -> p a", p=P))
        nc.sync.dma_start(bo_row[:], bo_d[None, :])
        nc.sync.dma_start(bvs_row[:], bvs_d[None, :])

        def bcast_row(row_ap, dst):  # (1,E) -> (128,E) via K=1 matmul
            ps = ps_big.tile([P, 1024], F32, tag="big")
            for c0, c1 in _chunks(E):
                nc.tensor.matmul(
                    ps[:, c0:c1], ones_r[:], row_ap[:, c0:c1],
                    start=True, stop=True,
                )
            nc.vector.tensor_copy(dst[:], ps[:, :E])

        aff_sb = {}
        for i, (gd, bd) in aff_d.items():
            g_row = persist.tile([1, E], BF16, tag=f"agr{i}")
            b_row = persist.tile([1, E], BF16, tag=f"abr{i}")
            nc.sync.dma_start(g_row[:], gd[None, :])
            nc.sync.dma_start(b_row[:], bd[None, :])
            g_t = persist.tile([P, E], F32, tag=f"ag{i}")
            b_t = persist.tile([P, E], F32, tag=f"ab{i}")
            bcast_row(g_row, g_t)
            bcast_row(b_row, b_t)
            aff_sb[i] = (g_t, b_t)

        # extra buffers for affine copies of residual-path tensors
        xn_res = xn
        if 0 in aff_sb:
            xn_res = persist.tile([P, TT, E], F32, tag="xna")

        qkp = ctx.enter_context(tc.tile_pool(name="qk", bufs=1))
        qT = qkp.tile([P, ET, T], BF16, tag="qT")
        kT = qkp.tile([P, ET, T], BF16, tag="kT")

        def layer_norm_tile(dst_ap, src_ap):
            """dst = (src - mean)/sqrt(var+eps), per-partition stats over E."""
            st6 = stp.tile([P, 2, 6], F32, tag="st6")
            half = src_ap.shape[-1] // 2
            nc.vector.bn_stats(st6[:, 0, :], src_ap[:, :half])
            nc.vector.bn_stats(st6[:, 1, :], src_ap[:, half:])
            mv = stp.tile([P, 2], F32, tag="mv")
            nc.vector.bn_aggr(mv[:], st6[:])
            std = stp.tile([P, 1], F32, tag="std")
            nc.scalar.activation(std[:], mv[:, 1:2], AF.Sqrt, bias=eps_col[:])
            rst = stp.tile([P, 1], F32, tag="rst")
            nc.vector.reciprocal(rst[:], std[:])
            nmr = stp.tile([P, 1], F32, tag="nmr")
            nc.vector.tensor_scalar(
                nmr[:], mv[:, 0:1], rst[:], -1.0, op0=OP.mult, op1=OP.mult
            )
            # big apply pass on ACT (idle at startup): x*rstd + (-mu*rstd)
            nc.scalar.activation(
                dst_ap, src_ap, AF.Identity, bias=nmr[:], scale=rst[:]
            )

        def affine_tile(dst_ap, src_ap, idx):
            g_t, b_t = aff_sb[idx]
            nc.vector.tensor_mul(dst_ap, src_ap, g_t[:])
            nc.vector.tensor_add(dst_ap, dst_ap, b_t[:])

        with tc.tile_pool(name="bc1", bufs=1) as bcp, \
             tc.tile_pool(name="io3", bufs=16) as iop, \
             tc.tile_pool(name="wpr", bufs=8) as wp, \
             tc.tile_pool(name="pstr", bufs=2, space="PSUM") as ps_tr:
            yn = bcp.tile([P, TT, E], F32, tag="yn")
            xnT = bcp.tile([P, ET, T], FP8, tag="xnT")

            # prefetch the whole x/y inputs: one long DMA stream up front
            # instead of per-tile loads serialized with the LN pipeline
            yin, xin = [], []
            for tt in range(TT):
                it = iop.tile([P, E], F32, tag="in")
                nc.sync.dma_start(it[:], y_d[tt * P:(tt + 1) * P, :])
                yin.append(it)
            for tt in range(TT):
                it = iop.tile([P, E], F32, tag="in")
                nc.sync.dma_start(it[:], x_d[tt * P:(tt + 1) * P, :])
                xin.append(it)

            # ~5us of junk matmuls on the identity tile: spins the PE HAM
            # activity window to full clock while the first input DMAs and
            # layer-norms run, so real matmuls start at 2.4 GHz
            warm = ps_tr.tile([P, 2, P], F32, tag="tr")
            for _ in range(16):
                nc.tensor.matmul(warm[:, 0, :], ident[:], ident[:],
                                 start=True, stop=True)

            # per-token-tile transpose into feature-major fp8 destination;
            # two 128x128 transposes share one PSUM bank; evacuations
            # alternate DVE/ACT to balance the element-wise engines
            def transpose_tile(dstT, src3, tt):
                for eh in range(ET // 2):
                    pst = ps_tr.tile([P, 2, P], F32, tag="tr")
                    for k in range(2):
                        et = 2 * eh + k
                        nc.tensor.transpose(
                            pst[:, k, :], src3[:, tt, et * P:(et + 1) * P],
                            ident[:]
                        )
                    dst = dstT[:, 2 * eh:2 * eh + 2, tt * P:(tt + 1) * P]
                    if eh == 0:
                        nc.vector.tensor_copy(dst, pst[:])
                    else:
                        nc.scalar.copy(dst, pst[:])

            def keep_hot(n=3):
                # PE p-state: the clock governor downclocks on idle; feed
                # junk matmuls during LN so projections start at 2.4 GHz
                w_ = ps_tr.tile([P, 2, P], F32, tag="tr")
                for _ in range(n):
                    nc.tensor.matmul(w_[:, 0, :], ident[:], ident[:],
                                     start=True, stop=True)

            def ln_y(tt):
                layer_norm_tile(yn[:, tt, :], yin[tt][:])
                keep_hot()
                if 1 in aff_sb:
                    ya = iop.tile([P, E], F32, tag="yaff")
                    affine_tile(ya[:], yn[:, tt, :], 1)
                    nc.sync.dma_start(oyn_d[tt * P:(tt + 1) * P, :], ya[:])
                else:
                    nc.sync.dma_start(oyn_d[tt * P:(tt + 1) * P, :], yn[:, tt, :])
                transpose_tile(ynT, yn, tt)

            def load_w(w_d, cols):
                # K-paired fp8 weight tiles: wt[p, i, e] = w[(2k+i)*128+p, e]
                wts = []
                for kp in range(KP):
                    wt = wp.tile([P, 2, cols], FP8, tag="w")
                    for i in range(2):
                        r0 = (2 * kp + i) * P
                        nc.sync.dma_start(wt[:, i, :], w_d[r0:r0 + P, :])
                    wts.append(wt)
                return wts

            def proj_chunk(wts, b_sb, outT, srcT, c0, c1, descale):
                for mt in range(outT.shape[1]):
                    ps = ps_big.tile([P, 1024], F32, tag="big")
                    for kp in range(KP):
                        nc.tensor.matmul(
                            ps[:, : c1 - c0],
                            wts[kp][:, :, mt * P:(mt + 1) * P],
                            srcT[:, 2 * kp:2 * kp + 2, c0:c1],
                            start=(kp == 0), stop=(kp == KP - 1),
                            perf_mode=DR,
                        )
                    nc.scalar.activation(
                        outT[:, mt, c0:c1], ps[:, : c1 - c0], AF.Identity,
                        bias=b_sb[:, mt:mt + 1], scale=descale,
                    )

            # interleave LN half-batches with projection chunks so PE has
            # dense matmul work while DVE runs the next LN batch
            def ln_x(tt):
                layer_norm_tile(xn[:, tt, :], xin[tt][:])
                keep_hot()
                if 0 in aff_sb:
                    affine_tile(xn_res[:, tt, :], xn[:, tt, :], 0)
                transpose_tile(xnT, xn, tt)

            for tt in range(TT // 2):
                ln_y(tt)
            wts_k = load_w(wk_d, E)
            proj_chunk(wts_k, bk_sb, kT, ynT, 0, T // 2, 1.0 / s_k)
            for tt in range(TT // 2, TT):
                ln_y(tt)
            proj_chunk(wts_k, bk_sb, kT, ynT, T // 2, T, 1.0 / s_k)
            for tt in range(TT // 2):
                ln_x(tt)
            wts_q = load_w(wq_d, E)
            proj_chunk(wts_q, bq_sb, qT, xnT, 0, T // 2, 1.0 / s_q)
            for tt in range(TT // 2, TT):
                ln_x(tt)
            proj_chunk(wts_q, bq_sb, qT, xnT, T // 2, T, 1.0 / s_q)
            # BV is first consumed by head 0's accumulate, well after this
            bcast_row(bvs_row, BV)
            bcast_row(bo_row, BO)

        # --- attention head loop ---
        with tc.tile_pool(name="wvp", bufs=14) as wvp, \
             tc.tile_pool(name="expp", bufs=18) as expp, \
             tc.tile_pool(name="vp", bufs=18) as vpp, \
             tc.tile_pool(name="rcp", bufs=8) as rcp, \
             tc.tile_pool(name="pssc", bufs=2, space="PSUM") as ps_sc:

            def v_mt(wvt, mt, vt):
                ps = ps_big.tile([P, 1024], F32, tag="big")
                for kp in range(KP):
                    for c0, c1 in _chunks(E):
                        nc.tensor.matmul(
                            ps[:, c0:c1],
                            ynT[:, 2 * kp:2 * kp + 2, mt * P:(mt + 1) * P],
                            wvt[kp][:, :, c0:c1],
                            start=(kp == 0), stop=(kp == KP - 1),
                            perf_mode=DR,
                        )
                v_ = vt[mt // 2]
                i = mt % 2
                nc.gpsimd.memset(v_[:, i, E:E + 1], 1.0)
                # descale (1/s_v) fused into the evacuation; alternate engines
                if mt % 2 == 0:
                    nc.vector.tensor_scalar_mul(
                        v_[:, i, :E], ps[:, :E], 1.0 / s_v
                    )
                else:
                    nc.scalar.mul(v_[:, i, :E], ps[:, :E], 1.0 / s_v)

            def t_nt(h, expt, vt, nt):
                ps = ps_big.tile([P, 1024], F32, tag="big")
                for kp in range(TP):
                    for c0, c1 in _chunks(E + 1):
                        nc.tensor.matmul(
                            ps[:, c0:c1],
                            expt[kp][:, :, nt * P:(nt + 1) * P],
                            vt[kp][:, :, c0:c1],
                            start=(kp == 0), stop=(kp == TP - 1),
                            perf_mode=DR,
                        )
                rc = rcp.tile([P, 1], F32, tag="rc")
                nc.vector.reciprocal(rc[:], ps[:, E:E + 1])
                prev = BV[:] if h == 0 else acc[:, nt, :]
                if h % 2 == 0:
                    nc.vector.scalar_tensor_tensor(
                        acc[:, nt, :], ps[:, :E], rc[:], prev,
                        op0=OP.mult, op1=OP.add,
                    )
                else:
                    # odd heads: normalize on ACT, accumulate on DVE --
                    # splits the work so neither engine paces the drain
                    tmp = rcp.tile([P, E], F32, tag="nrm")
                    nc.scalar.mul(tmp[:], ps[:, :E], rc[:])
                    nc.vector.tensor_add(acc[:, nt, :], tmp[:], prev)

            # heads in pairs: even head uses PE rows 0:64, odd head rows
            # 64:128 -> interleaved score matmuls run concurrently in the
            # array (distinct row groups). The pair loop is software-
            # pipelined: pair j's mt-steps carry both heads' v matmuls plus
            # pair j-1's normalize-accumulate, packing PE (scores+fp8 DR),
            # ACT (exp) and DVE (rc+stt) concurrently.
            prev_pair = None
            for j in range(H // 2):
                h0, h1 = 2 * j, 2 * j + 1
                wvt = {h0: [], h1: []}
                for h in (h0, h1):
                    for kp in range(KP):
                        wt = wvp.tile([P, 2, E], FP8, tag="wv")
                        for i in range(2):
                            r0 = (2 * kp + i) * P
                            nc.sync.dma_start(
                                wt[:, i, :], wv_d[h, r0:r0 + P, :]
                            )
                        wvt[h].append(wt)
                expt = {h0: [], h1: []}
                vt = {h0: [], h1: []}
                for h in (h0, h1):
                    for kp in range(TP):
                        ex = expp.tile([P, 2, T], FP8, tag="exp")
                        v_ = vpp.tile([P, 2, E + 4], FP8, tag="v")
                        expt[h].append(ex)
                        vt[h].append(v_)
                for mt in range(TT):
                    ps0 = ps_sc.tile([P, 1024], F32, tag="sc")
                    ps1 = ps_sc.tile([P, 1024], F32, tag="sc")
                    for c0, c1 in _chunks(T):
                        nc.tensor.matmul(
                            ps0[:, c0:c1],
                            kT[0:HD, j, mt * P:(mt + 1) * P],
                            qT[0:HD, j, c0:c1],
                            start=True, stop=True,
                        )
                        nc.tensor.matmul(
                            ps1[:, c0:c1],
                            kT[HD:P, j, mt * P:(mt + 1) * P],
                            qT[HD:P, j, c0:c1],
                            start=True, stop=True,
                        )
                    for h, ps in ((h0, ps0), (h1, ps1)):
                        nc.scalar.activation(
                            expt[h][mt // 2][:, mt % 2, :], ps[:, :T], AF.Exp
                        )
                    v_mt(wvt[h0], mt, vt[h0])
                    v_mt(wvt[h1], mt, vt[h1])
                    if prev_pair is not None:
                        p0, p1, pexp, pvt = prev_pair
                        t_nt(p0, pexp[p0], pvt[p0], mt)
                        t_nt(p1, pexp[p1], pvt[p1], mt)
                prev_pair = (h0, h1, expt, vt)
            p0, p1, pexp, pvt = prev_pair
            for nt in range(TT):
                t_nt(p0, pexp[p0], pvt[p0], nt)
                t_nt(p1, pexp[p1], pvt[p1], nt)

        # --- residual + LN3 + MLP ---
        with tc.tile_pool(name="mlp", bufs=1) as mp, \
             tc.tile_pool(name="out3", bufs=3) as op_, \
             tc.tile_pool(name="pstr2", bufs=2, space="PSUM") as ps_tr:
            hT = mp.tile([P, ET, T], FP8, tag="hT")
            ruT = mp.tile([P, ET, T], FP8, tag="ruT")
            h_plus = mp.tile([P, TT, E], F32, tag="hp")
            wi_sb = mp.tile([P, ET, E], FP8, tag="wi")
            wo_sb = mp.tile([P, ET, E], FP8, tag="wo")
            h_res = acc
            if 2 in aff_sb:
                h_res = mp.tile([P, TT, E], F32, tag="ha")
            nc.sync.dma_start(wi_sb[:], wi_d.rearrange("(a p) l -> p a l", p=P))
            nc.sync.dma_start(wo_sb[:], wo_d.rearrange("(a p) l -> p a l", p=P))

            def ln3(nt):
                nc.vector.tensor_add(
                    acc[:, nt, :], acc[:, nt, :], xn_res[:, nt, :]
                )
                layer_norm_tile(acc[:, nt, :], acc[:, nt, :])
                w_ = ps_tr.tile([P, 2, P], F32, tag="tr")
                for _ in range(2):
                    nc.tensor.matmul(w_[:, 0, :], ident[:], ident[:],
                                     start=True, stop=True)
                if 2 in aff_sb:
                    affine_tile(h_res[:, nt, :], acc[:, nt, :], 2)
                for eh in range(ET // 2):
                    pst = ps_tr.tile([P, 2, P], F32, tag="tr")
                    for k in range(2):
                        et = 2 * eh + k
                        nc.tensor.transpose(
                            pst[:, k, :], acc[:, nt, et * P:(et + 1) * P],
                            ident[:]
                        )
                    dst = hT[:, 2 * eh:2 * eh + 2, nt * P:(nt + 1) * P]
                    if eh % 2 == 0:
                        nc.vector.tensor_copy(dst, pst[:])
                    else:
                        nc.scalar.copy(dst, pst[:])
                # b_out pre-added on the idle Pool engine so the out loop
                # needs a single stt + DMA per token tile
                nc.gpsimd.tensor_add(
                    h_plus[:, nt, :], h_res[:, nt, :], BO[:]
                )

            def u_chunk(c0, c1):
                # u^T = relu(w_in^T @ hT + b_in), token-column chunk
                for mt in range(ET):
                    ps = ps_big.tile([P, 1024], F32, tag="big")
                    for kp in range(KP):
                        nc.tensor.matmul(
                            ps[:, : c1 - c0],
                            wi_sb[:, 2 * kp:2 * kp + 2, mt * P:(mt + 1) * P],
                            hT[:, 2 * kp:2 * kp + 2, c0:c1],
                            start=(kp == 0), stop=(kp == KP - 1),
                            perf_mode=DR,
                        )
                    nc.scalar.activation(
                        ruT[:, mt, c0:c1], ps[:, : c1 - c0], AF.Relu,
                        bias=bi_sb[:, mt:mt + 1], scale=1.0 / s_i,
                    )

            for nt in range(TT // 2):
                ln3(nt)
            u_chunk(0, T // 2)
            for nt in range(TT // 2, TT):
                ln3(nt)
            u_chunk(T // 2, T)
            # out1 = ruT^T @ w_out + b_out + h
            for nt in range(TT):
                ps = ps_big.tile([P, 1024], F32, tag="big")
                for kp in range(KP):
                    for c0, c1 in _chunks(E):
                        nc.tensor.matmul(
                            ps[:, c0:c1],
                            ruT[:, 2 * kp:2 * kp + 2, nt * P:(nt + 1) * P],
                            wo_sb[:, 2 * kp:2 * kp + 2, c0:c1],
                            start=(kp == 0), stop=(kp == KP - 1),
                            perf_mode=DR,
                        )
                ot = op_.tile([P, E], F32, tag="ot")
                nc.vector.scalar_tensor_tensor(
                    ot[:], ps[:, :E], 1.0 / s_o, h_plus[:, nt, :],
                    op0=OP.mult, op1=OP.add,
                )
                nc.sync.dma_start(o1_d[nt * P:(nt + 1) * P, :], ot[:])

    return nc


def _pow2_scale(w):
    """Power-of-2 scale putting max|w| into (64, 128] for fp8-e4m3."""
    m = float(np.max(np.abs(w)))
    if m == 0.0:
        return 1.0
    return float(2.0 ** np.floor(np.log2(128.0 / m)))


def host_prep(inputs, T, E, H):
    """Fold LN affines / scale / v-bias into weights (float64 on host)."""
    f8 = {k: np.asarray(v, np.float64) for k, v in inputs.items()}
    g1, b1 = f8["ln1_g"], f8["ln1_b"]
    g2, b2 = f8["ln2_g"], f8["ln2_b"]
    g3, b3 = f8["ln3_g"], f8["ln3_b"]
    scale = 1.0 / np.sqrt(HD)
    wq_f = (g1[:, None] * f8["wq"]) * scale
    bq_f = (b1 @ f8["wq"] + f8["bq"]) * scale
    wk_f = g2[:, None] * f8["wk"]
    bk_f = b2 @ f8["wk"] + f8["bk"]
    wv3 = f8["wv"].reshape(E, H, E)
    wv_f = np.ascontiguousarray((g2[:, None, None] * wv3).transpose(1, 0, 2))
    bvs = f8["bv"].reshape(H, E).sum(0) + b2 @ wv3.sum(axis=1)
    wi_f = g3[:, None] * f8["w_in"]
    bi_f = b3 @ f8["w_in"] + f8["b_in"]

    s_q = _pow2_scale(wq_f)
    s_k = _pow2_scale(wk_f)
    s_v = _pow2_scale(wv_f)
    s_i = _pow2_scale(wi_f)
    s_o = _pow2_scale(f8["w_out"])

    def ident_gate(g, b):
        return not (np.allclose(g, 1.0) and np.allclose(b, 0.0))

    aff = (ident_gate(g1, b1), ident_gate(g2, b2), ident_gate(g3, b3))
    w = {
        "wq": wq_f * s_q, "bq": bq_f, "wk": wk_f * s_k, "bk": bk_f,
        "wv": wv_f * s_v, "bvs": bvs,
        "w_in": wi_f * s_i, "b_in": bi_f,
        "w_out": f8["w_out"] * s_o, "b_out": f8["b_out"],
    }
    import ml_dtypes

    fp8_keys = {"wq", "wk", "wv", "w_in", "w_out"}
    bf16_keys = {"bvs", "b_out"}
    def cast(k, v):
        if k in fp8_keys:
            return np.ascontiguousarray(v, ml_dtypes.float8_e4m3)
        if k in bf16_keys:
            return np.ascontiguousarray(v, ml_dtypes.bfloat16)
        return np.ascontiguousarray(v, np.float32)

    w = {k: cast(k, v) for k, v in w.items()}
    for i, (g, b) in enumerate(((g1, b1), (g2, b2), (g3, b3))):
        if aff[i]:
            w[f"affg{i}"] = np.asarray(g, ml_dtypes.bfloat16)
            w[f"affb{i}"] = np.asarray(b, ml_dtypes.bfloat16)
    return w, aff, (s_q, s_k, s_v, s_i, s_o)


_NC_CACHE = {}


def _get_nc(T, E, H, aff, scales):
    key = (T, E, H, aff, scales)
    if key not in _NC_CACHE:
        nc = build(T, E, H, aff, *scales)
        nc.finalize()
        _NC_CACHE[key] = nc
    return _NC_CACHE[key]


def run(inputs, trace=False, tmpdir=None):
    from concourse.bass_utils import run_bass_kernel_spmd

    x = np.ascontiguousarray(np.asarray(inputs["x"], np.float32))
    y = np.ascontiguousarray(np.asarray(inputs["y"], np.float32))
    B, T, E = x.shape
    H = inputs["wv"].shape[1] // E
    assert B == N_CORES
    w, aff, scales = host_prep(inputs, T, E, H)
    nc = _get_nc(T, E, H, aff, scales)
    in_maps = [dict(w, x=x[c], y=y[c]) for c in range(B)]
    res = run_bass_kernel_spmd(
        nc, in_maps, core_ids=list(range(N_CORES)), trace=trace, tmpdir=tmpdir
    )
    o1 = np.stack([res.results[c]["o1"] for c in range(B)])
    oyn = np.stack([res.results[c]["oyn"] for c in range(B)])
    return (o1, oyn), res


def kernel(**inputs):
    (o1, oyn), _ = run(inputs)
    return (o1, oyn)


# revision 21
# speedup vs baseline: 1.0149x; 1.0149x over previous
"""Trainium2 Bass kernel for nn_Attention_33036888441230.

Cross-attention transformer block, B=8 batch sharded 1-per-core across 8
NeuronCores (pure data parallel, no collectives).

reference math (per batch):
  xn = LN(x,g1,b1); yn = LN(y,g2,b2)
  q = (xn@wq+bq).view(N,H,64); k = (yn@wk+bk).view(M,H,64)
  v = (yn@wv+bv).view(M,H,E)
  a = softmax(q.k^T/8, axis=m)
  dx = einsum('hnm,mhe->ne', a, v)       # heads summed
  h = LN3(xn + dx); out1 = h + relu(h@w_in+b_in)@w_out + b_out
  returns (out1, yn)

Implementation notes:
  - the four big matmul groups (q/k projections, v projection, attention.V
    accumulate) run in fp8-e4m3 with DoubleRow perf mode: K=256 per
    instruction at 2 weight-rows/PE-cell, 2-4x bf16 throughput. Weights are
    pre-scaled by a power of two on host into fp8's sweet range; the descale
    rides for free on the PSUM-evacuation instruction (ACT scale / DVE
    tensor_scalar imm).
  - score matmuls (K=64 per head) and the MLP stay bf16; layer-norm /
    softmax-normalization / residual math in fp32.
  - softmax has no max-subtraction (scores provably tiny); row-sums come for
    free from a ones-column appended to the attention.V matmul; 1/rowsum is
    fused into the per-head accumulate (scalar_tensor_tensor).
  - softmax-of-rows sums to 1 => v-bias contributes sum_h bv_h, folded on host.
  - LN gains/biases are folded into adjacent weights on host; the residual /
    output copies get an on-device affine only when non-identity.
  - PSUM evacuations are split between ACT and DVE (Pool cannot read PSUM)
    to balance the two element-wise engines.
  - heads processed in pairs: even head in PE rows 0:64, odd head in rows
    64:128, score matmuls interleaved so both run concurrently in the array.
"""

import sys

if "/opt/trn_rl_repo" not in sys.path:
    sys.path.insert(0, "/opt/trn_rl_repo")

from contextlib import ExitStack

import numpy as np

import concourse.bass as bass
import concourse.mybir as mybir
import concourse.tile as tile
from concourse import bacc
from concourse.masks import make_identity

F32 = mybir.dt.float32
BF16 = mybir.dt.bfloat16
FP8 = mybir.dt.float8e4
OP = mybir.AluOpType
AF = mybir.ActivationFunctionType
DR = mybir.MatmulPerfMode.DoubleRow

P = 128
HD = 64
EPS = 1e-5
N_CORES = 8


def _chunks(n, c=512):
    return [(i, min(i + c, n)) for i in range(0, n, c)]


def build(T, E, H, aff=(False, False, False), s_q=4096.0, s_k=2048.0,
          s_v=2048.0, s_i=2048.0, s_o=2048.0):
    """Build the per-core Bass graph. T tokens, E embed, H heads (HD=64)."""
    TT, ET = T // P, E // P
    KP = ET // 2  # fp8 DoubleRow K-pair count for E-contractions
    TP = TT // 2  # fp8 DoubleRow K-pair count for T-contractions
    assert H * HD == E
    nc = bacc.Bacc("TRN2", num_devices=N_CORES)

    x_d = nc.dram_tensor("x", [T, E], F32, kind="ExternalInput").ap()
    y_d = nc.dram_tensor("y", [T, E], F32, kind="ExternalInput").ap()
    wq_d = nc.dram_tensor("wq", [E, E], FP8, kind="ExternalInput").ap()
    wk_d = nc.dram_tensor("wk", [E, E], FP8, kind="ExternalInput").ap()
    wv_d = nc.dram_tensor("wv", [H, E, E], FP8, kind="ExternalInput").ap()
    wi_d = nc.dram_tensor("w_in", [E, E], FP8, kind="ExternalInput").ap()
    wo_d = nc.dram_tensor("w_out", [E, E], FP8, kind="ExternalInput").ap()
    bq_d = nc.dram_tensor("bq", [E], F32, kind="ExternalInput").ap()
    bk_d = nc.dram_tensor("bk", [E], F32, kind="ExternalInput").ap()
    bvs_d = nc.dram_tensor("bvs", [E], BF16, kind="ExternalInput").ap()
    bi_d = nc.dram_tensor("b_in", [E], F32, kind="ExternalInput").ap()
    bo_d = nc.dram_tensor("b_out", [E], BF16, kind="ExternalInput").ap()
    aff_d = {}
    for i, need in enumerate(aff):
        if need:
            aff_d[i] = (
                nc.dram_tensor(f"affg{i}", [E], BF16, kind="ExternalInput").ap(),
                nc.dram_tensor(f"affb{i}", [E], BF16, kind="ExternalInput").ap(),
            )
    o1_d = nc.dram_tensor("o1", [T, E], F32, kind="ExternalOutput").ap()
    oyn_d = nc.dram_tensor("oyn", [T, E], F32, kind="ExternalOutput").ap()

    with tile.TileContext(nc) as tc, ExitStack() as ctx:
        persist = ctx.enter_context(tc.tile_pool(name="persist", bufs=1))
        ps_big = ctx.enter_context(tc.tile_pool(name="psb", bufs=2, space="PSUM"))
        stp = ctx.enter_context(tc.tile_pool(name="stats", bufs=6))

        xn = persist.tile([P, TT, E], F32, tag="xn")
        acc = persist.tile([P, TT, E], F32, tag="acc")
        ynT = persist.tile([P, ET, T], FP8, tag="ynT")
        BV = persist.tile([P, E], F32, tag="BV")
        BO = persist.tile([P, E], F32, tag="BO")
        ident = persist.tile([P, P], F32, tag="ident")
        ones_r = persist.tile([1, P], BF16, tag="ones")
        bq_sb = persist.tile([P, ET], F32, tag="bq")
        bk_sb = persist.tile([P, ET], F32, tag="bk")
        bi_sb = persist.tile([P, ET], F32, tag="bi")
        bo_row = persist.tile([1, E], BF16, tag="bo")
        bvs_row = persist.tile([1, E], BF16, tag="bvs")
        eps_col = persist.tile([P, 1], F32, tag="eps")

        make_identity(nc, ident[:])
        nc.vector.memset(ones_r[:], 1.0)
        nc.vector.memset(eps_col[:], EPS)
        nc.sync.dma_start(bq_sb[:], bq_d.rearrange("(a p) -> p a", p=P))
        nc.sync.dma_start(bk_sb[:], bk_d.rearrange("(a p) -> p a", p=P))
        nc.sync.dma_start(bi_sb[:], bi_d.rearrange("(a p) -> p a", p=P))
        nc.sync.dma_start(bo_row[:], bo_d[None, :])
        nc.sync.dma_start(bvs_row[:], bvs_d[None, :])

        def bcast_row(row_ap, dst):  # (1,E) -> (128,E) via K=1 matmul
            ps = ps_big.tile([P, 1024], F32, tag="big")
            for c0, c1 in _chunks(E):
                nc.tensor.matmul(
                    ps[:, c0:c1], ones_r[:], row_ap[:, c0:c1],
                    start=True, stop=True,
                )
            nc.vector.tensor_copy(dst[:], ps[:, :E])

        aff_sb = {}
        for i, (gd, bd) in aff_d.items():
            g_row = persist.tile([1, E], BF16, tag=f"agr{i}")
            b_row = persist.tile([1, E], BF16, tag=f"abr{i}")
            nc.sync.dma_start(g_row[:], gd[None, :])
            nc.sync.dma_start(b_row[:], bd[None, :])
            g_t = persist.tile([P, E], F32, tag=f"ag{i}")
            b_t = persist.tile([P, E], F32, tag=f"ab{i}")
            bcast_row(g_row, g_t)
            bcast_row(b_row, b_t)
            aff_sb[i] = (g_t, b_t)

        # extra buffers for affine copies of residual-path tensors
        xn_res = xn
        if 0 in aff_sb:
            xn_res = persist.tile([P, TT, E], F32, tag="xna")

        qkp = ctx.enter_context(tc.tile_pool(name="qk", bufs=1))
        qT = qkp.tile([P, ET, T], BF16, tag="qT")
        kT = qkp.tile([P, ET, T], BF16, tag="kT")

        def layer_norm_tile(dst_ap, src_ap):
            """dst = (src - mean)/sqrt(var+eps), per-partition stats over E."""
            st6 = stp.tile([P, 2, 6], F32, tag="st6")
            half = src_ap.shape[-1] // 2
            nc.vector.bn_stats(st6[:, 0, :], src_ap[:, :half])
            nc.vector.bn_stats(st6[:, 1, :], src_ap[:, half:])
            mv = stp.tile([P, 2], F32, tag="mv")
            nc.vector.bn_aggr(mv[:], st6[:])
            std = stp.tile([P, 1], F32, tag="std")
            nc.scalar.activation(std[:], mv[:, 1:2], AF.Sqrt, bias=eps_col[:])
            rst = stp.tile([P, 1], F32, tag="rst")
            nc.vector.reciprocal(rst[:], std[:])
            nmr = stp.tile([P, 1], F32, tag="nmr")
            nc.vector.tensor_scalar(
                nmr[:], mv[:, 0:1], rst[:], -1.0, op0=OP.mult, op1=OP.mult
            )
            # big apply pass on ACT (idle at startup): x*rstd + (-mu*rstd)
            nc.scalar.activation(
                dst_ap, src_ap, AF.Identity, bias=nmr[:], scale=rst[:]
            )

        def affine_tile(dst_ap, src_ap, idx):
            g_t, b_t = aff_sb[idx]
            nc.vector.tensor_mul(dst_ap, src_ap, g_t[:])
            nc.vector.tensor_add(dst_ap, dst_ap, b_t[:])

        with tc.tile_pool(name="bc1", bufs=1) as bcp, \
             tc.tile_pool(name="io3", bufs=16) as iop, \
             tc.tile_pool(name="wpr", bufs=8) as wp, \
             tc.tile_pool(name="pstr", bufs=2, space="PSUM") as ps_tr:
            yn = bcp.tile([P, TT, E], F32, tag="yn")
            xnT = bcp.tile([P, ET, T], FP8, tag="xnT")

            # prefetch the whole x/y inputs: one long DMA stream up front
            # instead of per-tile loads serialized with the LN pipeline
            yin, xin = [], []
            for tt in range(TT):
                it = iop.tile([P, E], F32, tag="in")
                nc.sync.dma_start(it[:], y_d[tt * P:(tt + 1) * P, :])
                yin.append(it)
            for tt in range(TT):
                it = iop.tile([P, E], F32, tag="in")
                nc.sync.dma_start(it[:], x_d[tt * P:(tt + 1) * P, :])
                xin.append(it)

            # ~5us of junk matmuls on the identity tile: spins the PE HAM
            # activity window to full clock while the first input DMAs and
            # layer-norms run, so real matmuls start at 2.4 GHz
            warm = ps_tr.tile([P, 2, P], F32, tag="tr")
            for _ in range(16):
                nc.tensor.matmul(warm[:, 0, :], ident[:], ident[:],
                                 start=True, stop=True)

            # per-token-tile transpose into feature-major fp8 destination;
            # two 128x128 transposes share one PSUM bank; evacuations
            # alternate DVE/ACT to balance the element-wise engines
            def transpose_tile(dstT, src3, tt):
                for eh in range(ET // 2):
                    pst = ps_tr.tile([P, 2, P], F32, tag="tr")
                    for k in range(2):
                        et = 2 * eh + k
                        nc.tensor.transpose(
                            pst[:, k, :], src3[:, tt, et * P:(et + 1) * P],
                            ident[:]
                        )
                    dst = dstT[:, 2 * eh:2 * eh + 2, tt * P:(tt + 1) * P]
                    if eh % 2 == 0:
                        nc.vector.tensor_copy(dst, pst[:])
                    else:
                        nc.scalar.copy(dst, pst[:])

            def keep_hot(n=3):
                # PE p-state: the clock governor downclocks on idle; feed
                # junk matmuls during LN so projections start at 2.4 GHz
                w_ = ps_tr.tile([P, 2, P], F32, tag="tr")
                for _ in range(n):
                    nc.tensor.matmul(w_[:, 0, :], ident[:], ident[:],
                                     start=True, stop=True)

            def ln_y(tt):
                layer_norm_tile(yn[:, tt, :], yin[tt][:])
                keep_hot()
                if 1 in aff_sb:
                    ya = iop.tile([P, E], F32, tag="yaff")
                    affine_tile(ya[:], yn[:, tt, :], 1)
                    nc.sync.dma_start(oyn_d[tt * P:(tt + 1) * P, :], ya[:])
                else:
                    nc.sync.dma_start(oyn_d[tt * P:(tt + 1) * P, :], yn[:, tt, :])
                transpose_tile(ynT, yn, tt)

            def load_w(w_d, cols):
                # K-paired fp8 weight tiles: wt[p, i, e] = w[(2k+i)*128+p, e]
                wts = []
                for kp in range(KP):
                    wt = wp.tile([P, 2, cols], FP8, tag="w")
                    for i in range(2):
                        r0 = (2 * kp + i) * P
                        nc.sync.dma_start(wt[:, i, :], w_d[r0:r0 + P, :])
                    wts.append(wt)
                return wts

            def proj_chunk(wts, b_sb, outT, srcT, c0, c1, descale):
                for mt in range(outT.shape[1]):
                    ps = ps_big.tile([P, 1024], F32, tag="big")
                    for kp in range(KP):
                        nc.tensor.matmul(
                            ps[:, : c1 - c0],
                            wts[kp][:, :, mt * P:(mt + 1) * P],
                            srcT[:, 2 * kp:2 * kp + 2, c0:c1],
                            start=(kp == 0), stop=(kp == KP - 1),
                            perf_mode=DR,
                        )
                    nc.scalar.activation(
                        outT[:, mt, c0:c1], ps[:, : c1 - c0], AF.Identity,
                        bias=b_sb[:, mt:mt + 1], scale=descale,
                    )

            # interleave LN half-batches with projection chunks so PE has
            # dense matmul work while DVE runs the next LN batch
            def ln_x(tt):
                layer_norm_tile(xn[:, tt, :], xin[tt][:])
                keep_hot()
                if 0 in aff_sb:
                    affine_tile(xn_res[:, tt, :], xn[:, tt, :], 0)
                transpose_tile(xnT, xn, tt)

            for tt in range(TT // 2):
                ln_y(tt)
            wts_k = load_w(wk_d, E)
            proj_chunk(wts_k, bk_sb, kT, ynT, 0, T // 2, 1.0 / s_k)
            for tt in range(TT // 2, TT):
                ln_y(tt)
            proj_chunk(wts_k, bk_sb, kT, ynT, T // 2, T, 1.0 / s_k)
            for tt in range(TT // 2):
                ln_x(tt)
            wts_q = load_w(wq_d, E)
            proj_chunk(wts_q, bq_sb, qT, xnT, 0, T // 2, 1.0 / s_q)
            for tt in range(TT // 2, TT):
                ln_x(tt)
            proj_chunk(wts_q, bq_sb, qT, xnT, T // 2, T, 1.0 / s_q)
            # BV is first consumed by head 0's accumulate, well after this
            bcast_row(bvs_row, BV)
            bcast_row(bo_row, BO)

        # --- attention head loop ---
        with tc.tile_pool(name="wvp", bufs=14) as wvp, \
             tc.tile_pool(name="expp", bufs=18) as expp, \
             tc.tile_pool(name="vp", bufs=18) as vpp, \
             tc.tile_pool(name="rcp", bufs=8) as rcp, \
             tc.tile_pool(name="pssc", bufs=2, space="PSUM") as ps_sc:

            def v_mt(wvt, mt, vt):
                ps = ps_big.tile([P, 1024], F32, tag="big")
                for kp in range(KP):
                    for c0, c1 in _chunks(E):
                        nc.tensor.matmul(
                            ps[:, c0:c1],
                            ynT[:, 2 * kp:2 * kp + 2, mt * P:(mt + 1) * P],
                            wvt[kp][:, :, c0:c1],
                            start=(kp == 0), stop=(kp == KP - 1),
                            perf_mode=DR,
                        )
                v_ = vt[mt // 2]
                i = mt % 2
                nc.gpsimd.memset(v_[:, i, E:E + 1], 1.0)
                # descale (1/s_v) fused into the evacuation; alternate engines
                if mt % 2 == 0:
                    nc.vector.tensor_scalar_mul(
                        v_[:, i, :E], ps[:, :E], 1.0 / s_v
                    )
                else:
                    nc.scalar.mul(v_[:, i, :E], ps[:, :E], 1.0 / s_v)

            def t_nt(h, expt, vt, nt):
                ps = ps_big.tile([P, 1024], F32, tag="big")
                for kp in range(TP):
                    for c0, c1 in _chunks(E + 1):
                        nc.tensor.matmul(
                            ps[:, c0:c1],
                            expt[kp][:, :, nt * P:(nt + 1) * P],
                            vt[kp][:, :, c0:c1],
                            start=(kp == 0), stop=(kp == TP - 1),
                            perf_mode=DR,
                        )
                rc = rcp.tile([P, 1], F32, tag="rc")
                nc.vector.reciprocal(rc[:], ps[:, E:E + 1])
                prev = BV[:] if h == 0 else acc[:, nt, :]
                if h % 2 == 0:
                    nc.vector.scalar_tensor_tensor(
                        acc[:, nt, :], ps[:, :E], rc[:], prev,
                        op0=OP.mult, op1=OP.add,
                    )
                else:
                    # odd heads: normalize on ACT, accumulate on Pool --
                    # keeps DVE from pacing the drain
                    tmp = rcp.tile([P, E], F32, tag="nrm")
                    nc.scalar.mul(tmp[:], ps[:, :E], rc[:])
                    nc.gpsimd.tensor_add(acc[:, nt, :], tmp[:], prev)

            # heads in pairs: even head uses PE rows 0:64, odd head rows
            # 64:128 -> interleaved score matmuls run concurrently in the
            # array (distinct row groups). The pair loop is software-
            # pipelined: pair j's mt-steps carry both heads' v matmuls plus
            # pair j-1's normalize-accumulate, packing PE (scores+fp8 DR),
            # ACT (exp) and DVE (rc+stt) concurrently.
            prev_pair = None
            for j in range(H // 2):
                h0, h1 = 2 * j, 2 * j + 1
                wvt = {h0: [], h1: []}
                for h in (h0, h1):
                    for kp in range(KP):
                        wt = wvp.tile([P, 2, E], FP8, tag="wv")
                        for i in range(2):
                            r0 = (2 * kp + i) * P
                            nc.sync.dma_start(
                                wt[:, i, :], wv_d[h, r0:r0 + P, :]
                            )
                        wvt[h].append(wt)
                expt = {h0: [], h1: []}
                vt = {h0: [], h1: []}
                for h in (h0, h1):
                    for kp in range(TP):
                        ex = expp.tile([P, 2, T], FP8, tag="exp")
                        v_ = vpp.tile([P, 2, E + 4], FP8, tag="v")
                        expt[h].append(ex)
                        vt[h].append(v_)
                for mt in range(TT):
                    ps0 = ps_sc.tile([P, 1024], F32, tag="sc")
                    ps1 = ps_sc.tile([P, 1024], F32, tag="sc")
                    for c0, c1 in _chunks(T):
                        nc.tensor.matmul(
                            ps0[:, c0:c1],
                            kT[0:HD, j, mt * P:(mt + 1) * P],
                            qT[0:HD, j, c0:c1],
                            start=True, stop=True,
                        )
                        nc.tensor.matmul(
                            ps1[:, c0:c1],
                            kT[HD:P, j, mt * P:(mt + 1) * P],
                            qT[HD:P, j, c0:c1],
                            start=True, stop=True,
                        )
                    for h, ps in ((h0, ps0), (h1, ps1)):
                        nc.scalar.activation(
                            expt[h][mt // 2][:, mt % 2, :], ps[:, :T], AF.Exp
                        )
                    v_mt(wvt[h0], mt, vt[h0])
                    v_mt(wvt[h1], mt, vt[h1])
                    if prev_pair is not None:
                        p0, p1, pexp, pvt = prev_pair
                        t_nt(p0, pexp[p0], pvt[p0], mt)
                        t_nt(p1, pexp[p1], pvt[p1], mt)
                prev_pair = (h0, h1, expt, vt)
            p0, p1, pexp, pvt = prev_pair
            for nt in range(TT):
                t_nt(p0, pexp[p0], pvt[p0], nt)
                t_nt(p1, pexp[p1], pvt[p1], nt)

        # --- residual + LN3 + MLP ---
        with tc.tile_pool(name="mlp", bufs=1) as mp, \
             tc.tile_pool(name="out3", bufs=3) as op_, \
             tc.tile_pool(name="pstr2", bufs=2, space="PSUM") as ps_tr:
            hT = mp.tile([P, ET, T], FP8, tag="hT")
            ruT = mp.tile([P, ET, T], FP8, tag="ruT")
            h_plus = mp.tile([P, TT, E], F32, tag="hp")
            wi_sb = mp.tile([P, ET, E], FP8, tag="wi")
            wo_sb = mp.tile([P, ET, E], FP8, tag="wo")
            h_res = acc
            if 2 in aff_sb:
                h_res = mp.tile([P, TT, E], F32, tag="ha")
            nc.sync.dma_start(wi_sb[:], wi_d.rearrange("(a p) l -> p a l", p=P))
            nc.sync.dma_start(wo_sb[:], wo_d.rearrange("(a p) l -> p a l", p=P))

            def ln3(nt):
                nc.gpsimd.tensor_add(
                    acc[:, nt, :], acc[:, nt, :], xn_res[:, nt, :]
                )
                layer_norm_tile(acc[:, nt, :], acc[:, nt, :])
                w_ = ps_tr.tile([P, 2, P], F32, tag="tr")
                for _ in range(2):
                    nc.tensor.matmul(w_[:, 0, :], ident[:], ident[:],
                                     start=True, stop=True)
                if 2 in aff_sb:
                    affine_tile(h_res[:, nt, :], acc[:, nt, :], 2)
                for eh in range(ET // 2):
                    pst = ps_tr.tile([P, 2, P], F32, tag="tr")
                    for k in range(2):
                        et = 2 * eh + k
                        nc.tensor.transpose(
                            pst[:, k, :], acc[:, nt, et * P:(et + 1) * P],
                            ident[:]
                        )
                    dst = hT[:, 2 * eh:2 * eh + 2, nt * P:(nt + 1) * P]
                    if eh % 2 == 0:
                        nc.vector.tensor_copy(dst, pst[:])
                    else:
                        nc.scalar.copy(dst, pst[:])
                # b_out pre-added on the idle Pool engine so the out loop
                # needs a single stt + DMA per token tile
                nc.gpsimd.tensor_add(
                    h_plus[:, nt, :], h_res[:, nt, :], BO[:]
                )

            def u_chunk(c0, c1):
                # u^T = relu(w_in^T @ hT + b_in), token-column chunk
                for mt in range(ET):
                    ps = ps_big.tile([P, 1024], F32, tag="big")
                    for kp in range(KP):
                        nc.tensor.matmul(
                            ps[:, : c1 - c0],
                            wi_sb[:, 2 * kp:2 * kp + 2, mt * P:(mt + 1) * P],
                            hT[:, 2 * kp:2 * kp + 2, c0:c1],
                            start=(kp == 0), stop=(kp == KP - 1),
                            perf_mode=DR,
                        )
                    nc.scalar.activation(
                        ruT[:, mt, c0:c1], ps[:, : c1 - c0], AF.Relu,
                        bias=bi_sb[:, mt:mt + 1], scale=1.0 / s_i,
                    )

            for nt in range(TT // 2):
                ln3(nt)
            u_chunk(0, T // 2)
            for nt in range(TT // 2, TT):
                ln3(nt)
            u_chunk(T // 2, T)
            # out1 = ruT^T @ w_out + b_out + h
            for nt in range(TT):
                ps = ps_big.tile([P, 1024], F32, tag="big")
                for kp in range(KP):
                    for c0, c1 in _chunks(E):
                        nc.tensor.matmul(
                            ps[:, c0:c1],
                            ruT[:, 2 * kp:2 * kp + 2, nt * P:(nt + 1) * P],
                            wo_sb[:, 2 * kp:2 * kp + 2, c0:c1],
                            start=(kp == 0), stop=(kp == KP - 1),
                            perf_mode=DR,
                        )
                ot = op_.tile([P, E], F32, tag="ot")
                nc.vector.scalar_tensor_tensor(
                    ot[:], ps[:, :E], 1.0 / s_o, h_plus[:, nt, :],
                    op0=OP.mult, op1=OP.add,
                )
                nc.sync.dma_start(o1_d[nt * P:(nt + 1) * P, :], ot[:])

    return nc


def _pow2_scale(w):
    """Power-of-2 scale putting max|w| into (64, 128] for fp8-e4m3."""
    m = float(np.max(np.abs(w)))
    if m == 0.0:
        return 1.0
    return float(2.0 ** np.floor(np.log2(128.0 / m)))


def host_prep(inputs, T, E, H):
    """Fold LN affines / scale / v-bias into weights (float64 on host)."""
    f8 = {k: np.asarray(v, np.float64) for k, v in inputs.items()}
    g1, b1 = f8["ln1_g"], f8["ln1_b"]
    g2, b2 = f8["ln2_g"], f8["ln2_b"]
    g3, b3 = f8["ln3_g"], f8["ln3_b"]
    scale = 1.0 / np.sqrt(HD)
    wq_f = (g1[:, None] * f8["wq"]) * scale
    bq_f = (b1 @ f8["wq"] + f8["bq"]) * scale
    wk_f = g2[:, None] * f8["wk"]
    bk_f = b2 @ f8["wk"] + f8["bk"]
    wv3 = f8["wv"].reshape(E, H, E)
    wv_f = np.ascontiguousarray((g2[:, None, None] * wv3).transpose(1, 0, 2))
    bvs = f8["bv"].reshape(H, E).sum(0) + b2 @ wv3.sum(axis=1)
    wi_f = g3[:, None] * f8["w_in"]
    bi_f = b3 @ f8["w_in"] + f8["b_in"]

    s_q = _pow2_scale(wq_f)
    s_k = _pow2_scale(wk_f)
    s_v = _pow2_scale(wv_f)
    s_i = _pow2_scale(wi_f)
    s_o = _pow2_scale(f8["w_out"])

    def ident_gate(g, b):
        return not (np.allclose(g, 1.0) and np.allclose(b, 0.0))

    aff = (ident_gate(g1, b1), ident_gate(g2, b2), ident_gate(g3, b3))
    w = {
        "wq": wq_f * s_q, "bq": bq_f, "wk": wk_f * s_k, "bk": bk_f,
        "wv": wv_f * s_v, "bvs": bvs,
        "w_in": wi_f * s_i, "b_in": bi_f,
        "w_out": f8["w_out"] * s_o, "b_out": f8["b_out"],
    }
    import ml_dtypes

    fp8_keys = {"wq", "wk", "wv", "w_in", "w_out"}
    bf16_keys = {"bvs", "b_out"}
    def cast(k, v):
        if k in fp8_keys:
            return np.ascontiguousarray(v, ml_dtypes.float8_e4m3)
        if k in bf16_keys:
            return np.ascontiguousarray(v, ml_dtypes.bfloat16)
        return np.ascontiguousarray(v, np.float32)

    w = {k: cast(k, v) for k, v in w.items()}
    for i, (g, b) in enumerate(((g1, b1), (g2, b2), (g3, b3))):
        if aff[i]:
            w[f"affg{i}"] = np.asarray(g, ml_dtypes.bfloat16)
            w[f"affb{i}"] = np.asarray(b, ml_dtypes.bfloat16)
    return w, aff, (s_q, s_k, s_v, s_i, s_o)


_NC_CACHE = {}


def _get_nc(T, E, H, aff, scales):
    key = (T, E, H, aff, scales)
    if key not in _NC_CACHE:
        nc = build(T, E, H, aff, *scales)
        nc.finalize()
        _NC_CACHE[key] = nc
    return _NC_CACHE[key]


def run(inputs, trace=False, tmpdir=None):
    from concourse.bass_utils import run_bass_kernel_spmd

    x = np.ascontiguousarray(np.asarray(inputs["x"], np.float32))
    y = np.ascontiguousarray(np.asarray(inputs["y"], np.float32))
    B, T, E = x.shape
    H = inputs["wv"].shape[1] // E
    assert B == N_CORES
    w, aff, scales = host_prep(inputs, T, E, H)
    nc = _get_nc(T, E, H, aff, scales)
    in_maps = [dict(w, x=x[c], y=y[c]) for c in range(B)]
    res = run_bass_kernel_spmd(
        nc, in_maps, core_ids=list(range(N_CORES)), trace=trace, tmpdir=tmpdir
    )
    o1 = np.stack([res.results[c]["o1"] for c in range(B)])
    oyn = np.stack([res.results[c]["oyn"] for c in range(B)])
    return (o1, oyn), res


def kernel(**inputs):
    (o1, oyn), _ = run(inputs)
    return (o1, oyn)


# revision 22
# speedup vs baseline: 1.2234x; 1.2054x over previous
"""Trainium2 Bass kernel for nn_Attention_33036888441230.

Cross-attention transformer block, B=8 batch sharded 1-per-core across 8
NeuronCores (pure data parallel, no collectives).

reference math (per batch):
  xn = LN(x,g1,b1); yn = LN(y,g2,b2)
  q = (xn@wq+bq).view(N,H,64); k = (yn@wk+bk).view(M,H,64)
  v = (yn@wv+bv).view(M,H,E)
  a = softmax(q.k^T/8, axis=m)
  dx = einsum('hnm,mhe->ne', a, v)       # heads summed
  h = LN3(xn + dx); out1 = h + relu(h@w_in+b_in)@w_out + b_out
  returns (out1, yn)

Implementation notes:
  - the four big matmul groups (q/k projections, v projection, attention.V
    accumulate) run in fp8-e4m3 with DoubleRow perf mode: K=256 per
    instruction at 2 weight-rows/PE-cell, 2-4x bf16 throughput. Weights are
    pre-scaled by a power of two on host into fp8's sweet range; the descale
    rides for free on the PSUM-evacuation instruction (ACT scale / DVE
    tensor_scalar imm).
  - score matmuls (K=64 per head) and the MLP stay bf16; layer-norm /
    softmax-normalization / residual math in fp32.
  - softmax has no max-subtraction (scores provably tiny); row-sums come for
    free from a ones-column appended to the attention.V matmul; 1/rowsum is
    fused into the per-head accumulate (scalar_tensor_tensor).
  - softmax-of-rows sums to 1 => v-bias contributes sum_h bv_h, folded on host.
  - LN gains/biases are folded into adjacent weights on host; the residual /
    output copies get an on-device affine only when non-identity.
  - PSUM evacuations are split between ACT and DVE (Pool cannot read PSUM)
    to balance the two element-wise engines.
  - heads processed in pairs: even head in PE rows 0:64, odd head in rows
    64:128, score matmuls interleaved so both run concurrently in the array.
"""

import sys

if "/opt/trn_rl_repo" not in sys.path:
    sys.path.insert(0, "/opt/trn_rl_repo")

from contextlib import ExitStack

import numpy as np

import concourse.bass as bass
import concourse.mybir as mybir
import concourse.tile as tile
from concourse import bacc
from concourse.masks import make_identity

F32 = mybir.dt.float32
BF16 = mybir.dt.bfloat16
FP8 = mybir.dt.float8e4
OP = mybir.AluOpType
AF = mybir.ActivationFunctionType
DR = mybir.MatmulPerfMode.DoubleRow

P = 128
HD = 64
EPS = 1e-5
N_CORES = 8


def _chunks(n, c=512):
    return [(i, min(i + c, n)) for i in range(0, n, c)]


def build(T, E, H, aff=(False, False, False), s_q=4096.0, s_k=2048.0,
          s_v=2048.0, s_i=2048.0, s_o=2048.0):
    """Build the per-core Bass graph. T tokens, E embed, H heads (HD=64)."""
    TT, ET = T // P, E // P
    KP = ET // 2  # fp8 DoubleRow K-pair count for E-contractions
    TP = TT // 2  # fp8 DoubleRow K-pair count for T-contractions
    assert H * HD == E
    nc = bacc.Bacc("TRN2", num_devices=N_CORES)

    x_d = nc.dram_tensor("x", [T, E], F32, kind="ExternalInput").ap()
    y_d = nc.dram_tensor("y", [T, E], F32, kind="ExternalInput").ap()
    wq_d = nc.dram_tensor("wq", [E, E], FP8, kind="ExternalInput").ap()
    wk_d = nc.dram_tensor("wk", [E, E], FP8, kind="ExternalInput").ap()
    wv_d = nc.dram_tensor("wv", [H, E, E], FP8, kind="ExternalInput").ap()
    wi_d = nc.dram_tensor("w_in", [E, E], FP8, kind="ExternalInput").ap()
    wo_d = nc.dram_tensor("w_out", [E, E], FP8, kind="ExternalInput").ap()
    bq_d = nc.dram_tensor("bq", [E], F32, kind="ExternalInput").ap()
    bk_d = nc.dram_tensor("bk", [E], F32, kind="ExternalInput").ap()
    bvs_d = nc.dram_tensor("bvs", [E], BF16, kind="ExternalInput").ap()
    bi_d = nc.dram_tensor("b_in", [E], F32, kind="ExternalInput").ap()
    bo_d = nc.dram_tensor("b_out", [E], BF16, kind="ExternalInput").ap()
    aff_d = {}
    for i, need in enumerate(aff):
        if need:
            aff_d[i] = (
                nc.dram_tensor(f"affg{i}", [E], BF16, kind="ExternalInput").ap(),
                nc.dram_tensor(f"affb{i}", [E], BF16, kind="ExternalInput").ap(),
            )
    o1_d = nc.dram_tensor("o1", [T, E], F32, kind="ExternalOutput").ap()
    oyn_d = nc.dram_tensor("oyn", [T, E], F32, kind="ExternalOutput").ap()

    with tile.TileContext(nc) as tc, ExitStack() as ctx:
        persist = ctx.enter_context(tc.tile_pool(name="persist", bufs=1))
        ps_big = ctx.enter_context(tc.tile_pool(name="psb", bufs=2, space="PSUM"))
        stp = ctx.enter_context(tc.tile_pool(name="stats", bufs=6))

        xn = persist.tile([P, TT, E], F32, tag="xn")
        acc = persist.tile([P, TT, E], F32, tag="acc")
        ynT = persist.tile([P, ET, T], FP8, tag="ynT")
        BV = persist.tile([P, E], F32, tag="BV")
        BO = persist.tile([P, E], F32, tag="BO")
        ident = persist.tile([P, P], F32, tag="ident")
        ones_r = persist.tile([1, P], BF16, tag="ones")
        bq_sb = persist.tile([P, ET], F32, tag="bq")
        bk_sb = persist.tile([P, ET], F32, tag="bk")
        bi_sb = persist.tile([P, ET], F32, tag="bi")
        bo_row = persist.tile([1, E], BF16, tag="bo")
        bvs_row = persist.tile([1, E], BF16, tag="bvs")
        eps_col = persist.tile([P, 1], F32, tag="eps")

        make_identity(nc, ident[:])
        nc.vector.memset(ones_r[:], 1.0)
        nc.vector.memset(eps_col[:], EPS)
        nc.sync.dma_start(bq_sb[:], bq_d.rearrange("(a p) -> p a", p=P))
        nc.sync.dma_start(bk_sb[:], bk_d.rearrange("(a p) -> p a", p=P))
        nc.sync.dma_start(bi_sb[:], bi_d.rearrange("(a p) -> p a", p=P))
        nc.sync.dma_start(bo_row[:], bo_d[None, :])
        nc.sync.dma_start(bvs_row[:], bvs_d[None, :])

        def bcast_row(row_ap, dst):  # (1,E) -> (128,E) via K=1 matmul
            ps = ps_big.tile([P, 1024], F32, tag="big")
            for c0, c1 in _chunks(E):
                nc.tensor.matmul(
                    ps[:, c0:c1], ones_r[:], row_ap[:, c0:c1],
                    start=True, stop=True,
                )
            nc.vector.tensor_copy(dst[:], ps[:, :E])

        aff_sb = {}
        for i, (gd, bd) in aff_d.items():
            g_row = persist.tile([1, E], BF16, tag=f"agr{i}")
            b_row = persist.tile([1, E], BF16, tag=f"abr{i}")
            nc.sync.dma_start(g_row[:], gd[None, :])
            nc.sync.dma_start(b_row[:], bd[None, :])
            g_t = persist.tile([P, E], F32, tag=f"ag{i}")
            b_t = persist.tile([P, E], F32, tag=f"ab{i}")
            bcast_row(g_row, g_t)
            bcast_row(b_row, b_t)
            aff_sb[i] = (g_t, b_t)

        # extra buffers for affine copies of residual-path tensors
        xn_res = xn
        if 0 in aff_sb:
            xn_res = persist.tile([P, TT, E], F32, tag="xna")

        qkp = ctx.enter_context(tc.tile_pool(name="qk", bufs=1))
        qT = qkp.tile([P, ET, T], BF16, tag="qT")
        kT = qkp.tile([P, ET, T], BF16, tag="kT")

        def layer_norm_tile(dst_ap, src_ap):
            """dst = (src - mean)/sqrt(var+eps), per-partition stats over E."""
            st6 = stp.tile([P, 2, 6], F32, tag="st6")
            half = src_ap.shape[-1] // 2
            nc.vector.bn_stats(st6[:, 0, :], src_ap[:, :half])
            nc.vector.bn_stats(st6[:, 1, :], src_ap[:, half:])
            mv = stp.tile([P, 2], F32, tag="mv")
            nc.vector.bn_aggr(mv[:], st6[:])
            std = stp.tile([P, 1], F32, tag="std")
            nc.scalar.activation(std[:], mv[:, 1:2], AF.Sqrt, bias=eps_col[:])
            rst = stp.tile([P, 1], F32, tag="rst")
            nc.vector.reciprocal(rst[:], std[:])
            nmr = stp.tile([P, 1], F32, tag="nmr")
            nc.vector.tensor_scalar(
                nmr[:], mv[:, 0:1], rst[:], -1.0, op0=OP.mult, op1=OP.mult
            )
            # big apply pass on ACT (idle at startup): x*rstd + (-mu*rstd)
            nc.scalar.activation(
                dst_ap, src_ap, AF.Identity, bias=nmr[:], scale=rst[:]
            )

        def affine_tile(dst_ap, src_ap, idx):
            g_t, b_t = aff_sb[idx]
            nc.vector.tensor_mul(dst_ap, src_ap, g_t[:])
            nc.vector.tensor_add(dst_ap, dst_ap, b_t[:])

        with tc.tile_pool(name="bc1", bufs=1) as bcp, \
             tc.tile_pool(name="io3", bufs=16) as iop, \
             tc.tile_pool(name="wpr", bufs=8) as wp, \
             tc.tile_pool(name="pstr", bufs=2, space="PSUM") as ps_tr:
            yn = bcp.tile([P, TT, E], F32, tag="yn")
            xnT = bcp.tile([P, ET, T], FP8, tag="xnT")

            # prefetch the whole x/y inputs: one long DMA stream up front
            # instead of per-tile loads serialized with the LN pipeline
            yin, xin = [], []
            for tt in range(TT):
                it = iop.tile([P, E], F32, tag="in")
                nc.sync.dma_start(it[:], y_d[tt * P:(tt + 1) * P, :])
                yin.append(it)
            for tt in range(TT):
                it = iop.tile([P, E], F32, tag="in")
                nc.sync.dma_start(it[:], x_d[tt * P:(tt + 1) * P, :])
                xin.append(it)

            # ~5us of junk matmuls on the identity tile: spins the PE HAM
            # activity window to full clock while the first input DMAs and
            # layer-norms run, so real matmuls start at 2.4 GHz
            warm = ps_tr.tile([P, 2, P], F32, tag="tr")
            for _ in range(16):
                nc.tensor.matmul(warm[:, 0, :], ident[:], ident[:],
                                 start=True, stop=True)

            # per-token-tile transpose into feature-major fp8 destination;
            # two 128x128 transposes share one PSUM bank; evacuations
            # alternate DVE/ACT to balance the element-wise engines
            def transpose_tile(dstT, src3, tt):
                for eh in range(ET // 2):
                    pst = ps_tr.tile([P, 2, P], F32, tag="tr")
                    for k in range(2):
                        et = 2 * eh + k
                        nc.tensor.transpose(
                            pst[:, k, :], src3[:, tt, et * P:(et + 1) * P],
                            ident[:]
                        )
                    dst = dstT[:, 2 * eh:2 * eh + 2, tt * P:(tt + 1) * P]
                    if eh % 2 == 0:
                        nc.vector.tensor_copy(dst, pst[:])
                    else:
                        nc.scalar.copy(dst, pst[:])

            def keep_hot(n=3):
                # PE p-state: the clock governor downclocks on idle; feed
                # junk matmuls during LN so projections start at 2.4 GHz
                w_ = ps_tr.tile([P, 2, P], F32, tag="tr")
                for _ in range(n):
                    nc.tensor.matmul(w_[:, 0, :], ident[:], ident[:],
                                     start=True, stop=True)

            def ln_y(tt):
                layer_norm_tile(yn[:, tt, :], yin[tt][:])
                keep_hot()
                if 1 in aff_sb:
                    ya = iop.tile([P, E], F32, tag="yaff")
                    affine_tile(ya[:], yn[:, tt, :], 1)
                    nc.sync.dma_start(oyn_d[tt * P:(tt + 1) * P, :], ya[:])
                else:
                    nc.sync.dma_start(oyn_d[tt * P:(tt + 1) * P, :], yn[:, tt, :])
                transpose_tile(ynT, yn, tt)

            def load_w(w_d, cols):
                # K-paired fp8 weight tiles: wt[p, i, e] = w[(2k+i)*128+p, e]
                wts = []
                for kp in range(KP):
                    wt = wp.tile([P, 2, cols], FP8, tag="w")
                    for i in range(2):
                        r0 = (2 * kp + i) * P
                        nc.sync.dma_start(wt[:, i, :], w_d[r0:r0 + P, :])
                    wts.append(wt)
                return wts

            def proj_chunk(wts, b_sb, outT, srcT, c0, c1, descale):
                for mt in range(outT.shape[1]):
                    ps = ps_big.tile([P, 1024], F32, tag="big")
                    for kp in range(KP):
                        nc.tensor.matmul(
                            ps[:, : c1 - c0],
                            wts[kp][:, :, mt * P:(mt + 1) * P],
                            srcT[:, 2 * kp:2 * kp + 2, c0:c1],
                            start=(kp == 0), stop=(kp == KP - 1),
                            perf_mode=DR,
                        )
                    nc.scalar.activation(
                        outT[:, mt, c0:c1], ps[:, : c1 - c0], AF.Identity,
                        bias=b_sb[:, mt:mt + 1], scale=descale,
                    )

            # interleave LN half-batches with projection chunks so PE has
            # dense matmul work while DVE runs the next LN batch
            def ln_x(tt):
                layer_norm_tile(xn[:, tt, :], xin[tt][:])
                keep_hot()
                if 0 in aff_sb:
                    affine_tile(xn_res[:, tt, :], xn[:, tt, :], 0)
                transpose_tile(xnT, xn, tt)

            for tt in range(TT // 2):
                ln_y(tt)
            wts_k = load_w(wk_d, E)
            proj_chunk(wts_k, bk_sb, kT, ynT, 0, T // 2, 1.0 / s_k)
            for tt in range(TT // 2, TT):
                ln_y(tt)
            proj_chunk(wts_k, bk_sb, kT, ynT, T // 2, T, 1.0 / s_k)
            for tt in range(TT // 2):
                ln_x(tt)
            wts_q = load_w(wq_d, E)
            proj_chunk(wts_q, bq_sb, qT, xnT, 0, T // 2, 1.0 / s_q)
            for tt in range(TT // 2, TT):
                ln_x(tt)
            proj_chunk(wts_q, bq_sb, qT, xnT, T // 2, T, 1.0 / s_q)
            # BV is first consumed by head 0's accumulate, well after this
            bcast_row(bvs_row, BV)
            bcast_row(bo_row, BO)

        # --- attention head loop ---
        with tc.tile_pool(name="wvp", bufs=14) as wvp, \
             tc.tile_pool(name="expp", bufs=18) as expp, \
             tc.tile_pool(name="vp", bufs=18) as vpp, \
             tc.tile_pool(name="rcp", bufs=8) as rcp, \
             tc.tile_pool(name="pssc", bufs=2, space="PSUM") as ps_sc:

            def v_mt(wvt, mt, vt):
                ps = ps_big.tile([P, 1024], F32, tag="big")
                for kp in range(KP):
                    for c0, c1 in _chunks(E):
                        nc.tensor.matmul(
                            ps[:, c0:c1],
                            ynT[:, 2 * kp:2 * kp + 2, mt * P:(mt + 1) * P],
                            wvt[kp][:, :, c0:c1],
                            start=(kp == 0), stop=(kp == KP - 1),
                            perf_mode=DR,
                        )
                v_ = vt[mt // 2]
                i = mt % 2
                nc.gpsimd.memset(v_[:, i, E:E + 1], 1.0)
                # descale (1/s_v) fused into the evacuation; alternate engines
                if mt % 2 == 0:
                    nc.vector.tensor_scalar_mul(
                        v_[:, i, :E], ps[:, :E], 1.0 / s_v
                    )
                else:
                    nc.scalar.mul(v_[:, i, :E], ps[:, :E], 1.0 / s_v)

            def t_nt(h, expt, vt, nt):
                ps = ps_big.tile([P, 1024], F32, tag="big")
                for kp in range(TP):
                    for c0, c1 in _chunks(E + 1):
                        nc.tensor.matmul(
                            ps[:, c0:c1],
                            expt[kp][:, :, nt * P:(nt + 1) * P],
                            vt[kp][:, :, c0:c1],
                            start=(kp == 0), stop=(kp == TP - 1),
                            perf_mode=DR,
                        )
                rc = rcp.tile([P, 1], F32, tag="rc")
                nc.vector.reciprocal(rc[:], ps[:, E:E + 1])
                prev = BV[:] if h == 0 else acc[:, nt, :]
                if h % 2 == 0:
                    nc.vector.scalar_tensor_tensor(
                        acc[:, nt, :], ps[:, :E], rc[:], prev,
                        op0=OP.mult, op1=OP.add,
                    )
                else:
                    # odd heads: normalize on ACT, accumulate on Pool --
                    # keeps DVE from pacing the drain
                    tmp = rcp.tile([P, E], F32, tag="nrm")
                    nc.scalar.mul(tmp[:], ps[:, :E], rc[:])
                    nc.gpsimd.tensor_add(acc[:, nt, :], tmp[:], prev)

            # heads in pairs: even head uses PE rows 0:64, odd head rows
            # 64:128 -> interleaved score matmuls run concurrently in the
            # array (distinct row groups). The pair loop is software-
            # pipelined: pair j's mt-steps carry both heads' v matmuls plus
            # pair j-1's normalize-accumulate, packing PE (scores+fp8 DR),
            # ACT (exp) and DVE (rc+stt) concurrently.
            prev_pair = None
            for j in range(H // 2):
                h0, h1 = 2 * j, 2 * j + 1
                wvt = {h0: [], h1: []}
                for h in (h0, h1):
                    for kp in range(KP):
                        wt = wvp.tile([P, 2, E], FP8, tag="wv")
                        for i in range(2):
                            r0 = (2 * kp + i) * P
                            nc.sync.dma_start(
                                wt[:, i, :], wv_d[h, r0:r0 + P, :]
                            )
                        wvt[h].append(wt)
                expt = {h0: [], h1: []}
                vt = {h0: [], h1: []}
                for h in (h0, h1):
                    for kp in range(TP):
                        ex = expp.tile([P, 2, T], FP8, tag="exp")
                        v_ = vpp.tile([P, 2, E + 4], FP8, tag="v")
                        expt[h].append(ex)
                        vt[h].append(v_)
                for mt in range(TT):
                    ps0 = ps_sc.tile([P, 1024], F32, tag="sc")
                    ps1 = ps_sc.tile([P, 1024], F32, tag="sc")
                    for c0, c1 in _chunks(T):
                        nc.tensor.matmul(
                            ps0[:, c0:c1],
                            kT[0:HD, j, mt * P:(mt + 1) * P],
                            qT[0:HD, j, c0:c1],
                            start=True, stop=True,
                        )
                        nc.tensor.matmul(
                            ps1[:, c0:c1],
                            kT[HD:P, j, mt * P:(mt + 1) * P],
                            qT[HD:P, j, c0:c1],
                            start=True, stop=True,
                        )
                    for h, ps in ((h0, ps0), (h1, ps1)):
                        nc.scalar.activation(
                            expt[h][mt // 2][:, mt % 2, :], ps[:, :T], AF.Exp
                        )
                    v_mt(wvt[h0], mt, vt[h0])
                    v_mt(wvt[h1], mt, vt[h1])
                    if prev_pair is not None:
                        p0, p1, pexp, pvt = prev_pair
                        t_nt(p0, pexp[p0], pvt[p0], mt)
                        t_nt(p1, pexp[p1], pvt[p1], mt)
                prev_pair = (h0, h1, expt, vt)
            p0, p1, pexp, pvt = prev_pair
            for nt in range(TT):
                t_nt(p0, pexp[p0], pvt[p0], nt)
                t_nt(p1, pexp[p1], pvt[p1], nt)

        # --- residual + LN3 + MLP ---
        with tc.tile_pool(name="mlp", bufs=1) as mp, \
             tc.tile_pool(name="out3", bufs=3) as op_, \
             tc.tile_pool(name="pstr2", bufs=2, space="PSUM") as ps_tr:
            hT = mp.tile([P, ET, T], FP8, tag="hT")
            ruT = mp.tile([P, ET, T], FP8, tag="ruT")
            wi_sb = mp.tile([P, ET, E], FP8, tag="wi")
            wo_sb = mp.tile([P, ET, E], FP8, tag="wo")
            h_res = acc
            if 2 in aff_sb:
                h_res = mp.tile([P, TT, E], F32, tag="ha")
            nc.sync.dma_start(wi_sb[:], wi_d.rearrange("(a p) l -> p a l", p=P))
            nc.sync.dma_start(wo_sb[:], wo_d.rearrange("(a p) l -> p a l", p=P))

            def ln3(nt):
                nc.gpsimd.tensor_add(
                    acc[:, nt, :], acc[:, nt, :], xn_res[:, nt, :]
                )
                layer_norm_tile(acc[:, nt, :], acc[:, nt, :])
                w_ = ps_tr.tile([P, 2, P], F32, tag="tr")
                for _ in range(2):
                    nc.tensor.matmul(w_[:, 0, :], ident[:], ident[:],
                                     start=True, stop=True)
                if 2 in aff_sb:
                    affine_tile(h_res[:, nt, :], acc[:, nt, :], 2)
                for eh in range(ET // 2):
                    pst = ps_tr.tile([P, 2, P], F32, tag="tr")
                    for k in range(2):
                        et = 2 * eh + k
                        nc.tensor.transpose(
                            pst[:, k, :], acc[:, nt, et * P:(et + 1) * P],
                            ident[:]
                        )
                    dst = hT[:, 2 * eh:2 * eh + 2, nt * P:(nt + 1) * P]
                    if eh % 2 == 0:
                        nc.vector.tensor_copy(dst, pst[:])
                    else:
                        nc.scalar.copy(dst, pst[:])

            def u_chunk(c0, c1):
                # u^T = relu(w_in^T @ hT + b_in), token-column chunk
                for mt in range(ET):
                    ps = ps_big.tile([P, 1024], F32, tag="big")
                    for kp in range(KP):
                        nc.tensor.matmul(
                            ps[:, : c1 - c0],
                            wi_sb[:, 2 * kp:2 * kp + 2, mt * P:(mt + 1) * P],
                            hT[:, 2 * kp:2 * kp + 2, c0:c1],
                            start=(kp == 0), stop=(kp == KP - 1),
                            perf_mode=DR,
                        )
                    nc.scalar.activation(
                        ruT[:, mt, c0:c1], ps[:, : c1 - c0], AF.Relu,
                        bias=bi_sb[:, mt:mt + 1], scale=1.0 / s_i,
                    )

            for nt in range(TT // 2):
                ln3(nt)
            u_chunk(0, T // 2)
            for nt in range(TT // 2, TT):
                ln3(nt)
            u_chunk(T // 2, T)
            # out1 = ruT^T @ w_out + b_out + h
            for nt in range(TT):
                ps = ps_big.tile([P, 1024], F32, tag="big")
                for kp in range(KP):
                    for c0, c1 in _chunks(E):
                        nc.tensor.matmul(
                            ps[:, c0:c1],
                            ruT[:, 2 * kp:2 * kp + 2, nt * P:(nt + 1) * P],
                            wo_sb[:, 2 * kp:2 * kp + 2, c0:c1],
                            start=(kp == 0), stop=(kp == KP - 1),
                            perf_mode=DR,
                        )
                ot = op_.tile([P, E], F32, tag="ot")
                nc.vector.scalar_tensor_tensor(
                    ot[:], ps[:, :E], 1.0 / s_o, h_res[:, nt, :],
                    op0=OP.mult, op1=OP.add,
                )
                nc.gpsimd.tensor_add(ot[:], ot[:], BO[:])
                nc.sync.dma_start(o1_d[nt * P:(nt + 1) * P, :], ot[:])

    return nc


def _pow2_scale(w):
    """Power-of-2 scale putting max|w| into (64, 128] for fp8-e4m3."""
    m = float(np.max(np.abs(w)))
    if m == 0.0:
        return 1.0
    return float(2.0 ** np.floor(np.log2(128.0 / m)))


def host_prep(inputs, T, E, H):
    """Fold LN affines / scale / v-bias into weights (float64 on host)."""
    f8 = {k: np.asarray(v, np.float64) for k, v in inputs.items()}
    g1, b1 = f8["ln1_g"], f8["ln1_b"]
    g2, b2 = f8["ln2_g"], f8["ln2_b"]
    g3, b3 = f8["ln3_g"], f8["ln3_b"]
    scale = 1.0 / np.sqrt(HD)
    wq_f = (g1[:, None] * f8["wq"]) * scale
    bq_f = (b1 @ f8["wq"] + f8["bq"]) * scale
    wk_f = g2[:, None] * f8["wk"]
    bk_f = b2 @ f8["wk"] + f8["bk"]
    wv3 = f8["wv"].reshape(E, H, E)
    wv_f = np.ascontiguousarray((g2[:, None, None] * wv3).transpose(1, 0, 2))
    bvs = f8["bv"].reshape(H, E).sum(0) + b2 @ wv3.sum(axis=1)
    wi_f = g3[:, None] * f8["w_in"]
    bi_f = b3 @ f8["w_in"] + f8["b_in"]

    s_q = _pow2_scale(wq_f)
    s_k = _pow2_scale(wk_f)
    s_v = _pow2_scale(wv_f)
    s_i = _pow2_scale(wi_f)
    s_o = _pow2_scale(f8["w_out"])

    def ident_gate(g, b):
        return not (np.allclose(g, 1.0) and np.allclose(b, 0.0))

    aff = (ident_gate(g1, b1), ident_gate(g2, b2), ident_gate(g3, b3))
    w = {
        "wq": wq_f * s_q, "bq": bq_f, "wk": wk_f * s_k, "bk": bk_f,
        "wv": wv_f * s_v, "bvs": bvs,
        "w_in": wi_f * s_i, "b_in": bi_f,
        "w_out": f8["w_out"] * s_o, "b_out": f8["b_out"],
    }
    import ml_dtypes

    fp8_keys = {"wq", "wk", "wv", "w_in", "w_out"}
    bf16_keys = {"bvs", "b_out"}
    def cast(k, v):
        if k in fp8_keys:
            return np.ascontiguousarray(v, ml_dtypes.float8_e4m3)
        if k in bf16_keys:
            return np.ascontiguousarray(v, ml_dtypes.bfloat16)
        return np.ascontiguousarray(v, np.float32)

    w = {k: cast(k, v) for k, v in w.items()}
    for i, (g, b) in enumerate(((g1, b1), (g2, b2), (g3, b3))):
        if aff[i]:
            w[f"affg{i}"] = np.asarray(g, ml_dtypes.bfloat16)
            w[f"affb{i}"] = np.asarray(b, ml_dtypes.bfloat16)
    return w, aff, (s_q, s_k, s_v, s_i, s_o)


_NC_CACHE = {}


def _get_nc(T, E, H, aff, scales):
    key = (T, E, H, aff, scales)
    if key not in _NC_CACHE:
        nc = build(T, E, H, aff, *scales)
        nc.finalize()
        _NC_CACHE[key] = nc
    return _NC_CACHE[key]


def run(inputs, trace=False, tmpdir=None):
    from concourse.bass_utils import run_bass_kernel_spmd

    x = np.ascontiguousarray(np.asarray(inputs["x"], np.float32))
    y = np.ascontiguousarray(np.asarray(inputs["y"], np.float32))
    B, T, E = x.shape
    H = inputs["wv"].shape[1] // E
    assert B == N_CORES
    w, aff, scales = host_prep(inputs, T, E, H)
    nc = _get_nc(T, E, H, aff, scales)
    in_maps = [dict(w, x=x[c], y=y[c]) for c in range(B)]
    res = run_bass_kernel_spmd(
        nc, in_maps, core_ids=list(range(N_CORES)), trace=trace, tmpdir=tmpdir
    )
    o1 = np.stack([res.results[c]["o1"] for c in range(B)])
    oyn = np.stack([res.results[c]["oyn"] for c in range(B)])
    return (o1, oyn), res


def kernel(**inputs):
    (o1, oyn), _ = run(inputs)
    return (o1, oyn)
